# revision 56
# baseline (speedup 1.0000x reference)
"""Bahdanau attention Trainium2 kernel.

Math: the reference computes
    energy[b,s,:] = h[b] @ Wh.T + enc[b,s] @ We.T + bias          (Wh=W[:,:H], We=W[:,H:])
    attn_energies[b,s] = energy[b,s,:] @ v
    attn_weights = softmax_s(attn_energies)
    context[b] = attn_weights[b] @ enc[b]

Because attn_energies[b,s] = h[b]@(Wh.T@v) + enc[b,s]@(We.T@v) + bias@v and the
first/last terms are constant in s, they cancel inside the softmax.  So

    attn_weights[b] = softmax_s(enc[b] @ u),   u = We.T @ v   (computed on device)

which avoids the [B,S,H] energy tensor and its dense matmul entirely.  The
kernel streams each example's encoder block through SBUF once; per 128-row
chunk: DVE fused multiply-reduce against u, unshifted exp on ACT (max|e| ~ 66
so fp32 exp cannot overflow), and PE accumulation of the unnormalized
weighted sum, with 1/Z folded in at the end.  Energies and the softmax run
in full fp32; only the context weighted-sum matmul uses fp32r (TF32-like,
4x PE throughput), which costs ~2e-4 relative error on the context output.
To revert to full fp32 (~1.9e-5 error, ~8% slower): matmul on `t`/`exp_tile`
instead of `t_r`/`expr_tile` and drop the rounding copies.

Sharding: data-parallel over batch, 4 examples per core on 8 cores.
The only cross-core traffic is a 512B-per-rank AllGather of the u shards
(each core loads 1/8th of We instead of all 4MB of it).
"""

import numpy as np

B, S, H = 32, 2048, 1024
N_CORES = 8
B_LOC = B // N_CORES           # 4 examples per core
P = 128                        # SBUF partitions
S_CHUNKS = S // P              # 16 chunks of 128 positions
E_CHUNKS = H // P              # 8 chunks of the energy/contraction dim
NH = 512                       # matmul moving-free-dim limit

_CACHE = {}


def _build_bass():
    from contextlib import ExitStack

    import concourse.bacc as bacc
    import concourse.bass as bass
    import concourse.mybir as mybir
    from concourse.masks import make_identity
    from concourse.tile import TileContext

    f32 = mybir.dt.float32
    f32r = mybir.dt.float32r
    nc = bacc.Bacc(None, target_bir_lowering=False)

    enc_d = nc.declare_dram_parameter("enc", [B_LOC, S, H], f32, isOutput=False)
    w_d = nc.declare_dram_parameter("wslice", [H, P], f32, isOutput=False)
    v_d = nc.declare_dram_parameter("v", [H, 1], f32, isOutput=False)
    ctx_d = nc.declare_dram_parameter("ctx", [B_LOC, H], f32, isOutput=True)
    attnw_d = nc.declare_dram_parameter("attnw", [B_LOC, S_CHUNKS, P], f32, isOutput=True)

    with TileContext(nc) as tc, ExitStack() as ctx:
        const = ctx.enter_context(tc.tile_pool(name="const", bufs=1))
        wpool = ctx.enter_context(tc.tile_pool(name="wpool", bufs=1))

        ident = const.tile([P, P], f32)
        make_identity(nc, ident)
        ones_col = const.tile([P, S_CHUNKS], f32)
        nc.vector.memset(ones_col, 1.0)

        # v free-broadcast: v_bc[p, c, m] = v[c*128 + p] for all m, so a
        # single matmul chain produces u already broadcast across partitions:
        # ub[m, n] = sum_e v_bc[e, c, m] * We[e, n] = u[n] for every m.
        # (step-0 DMA APs are rejected by walrus, so broadcast on the DVE.)
        ones_sq = const.tile([P, P], f32)
        nc.vector.memset(ones_sq, 1.0)
        v_sb = const.tile([P, E_CHUNKS, 1], f32)
        nc.sync.dma_start(out=v_sb, in_=v_d[:, :].rearrange("(c p) o -> p c o", p=P))
        v_bc = const.tile([P, E_CHUNKS, P], f32)
        for c in range(E_CHUNKS):
            nc.vector.tensor_scalar_mul(
                v_bc[:, c, :], in0=ones_sq, scalar1=v_sb[:, c, :]
            )

        # Each core loads only ITS 128-column slice of We (512KB instead of
        # 4MB) and computes its u shard; a tiny AllGather (512B/rank) shares
        # the shards. wsl_sb[p, c, h'] = wslice[c*128 + p, h'].
        wsl_sb = wpool.tile([P, E_CHUNKS, P], f32)
        CG = E_CHUNKS // 2
        for g in range(2):
            nc.sync.dma_start(
                out=wsl_sb[:, g * CG:(g + 1) * CG, :],
                in_=w_d[g * CG * P:(g + 1) * CG * P, :].rearrange(
                    "(c p) h -> p c h", p=P),
            )

        # u_i[h'] = sum_e v[e] * wslice[e, h'] on this core, AllGather the 8
        # shards into u [1024], then broadcast across partitions on the PE.
        ones_row = const.tile([1, P], f32)
        nc.vector.memset(ones_row, 1.0)
        u_rep = const.tile([P, H], f32)
        u_sb = const.tile([1, H], f32)
        ui_sb = const.tile([1, P], f32)
        warm = const.tile([P, NH], f32)
        nc.vector.memset(warm, 0.0)
        dramp = ctx.enter_context(tc.tile_pool(name="dramp", bufs=1, space="DRAM"))
        cc_in = dramp.tile([1, P], f32)
        cc_out = dramp.tile([1, H], f32, addr_space="Shared")
        with tc.tile_pool(name="psu", bufs=1, space="PSUM") as psu:
            # Keep the PE continuously busy while the slice streams in, so the
            # p-state ramp reaches full clock before the real u matmuls.
            warm_ps = psu.tile([P, NH], f32)
            for _ in range(1):
                nc.tensor.matmul(warm_ps, lhsT=ident, rhs=warm,
                                 start=True, stop=True)
            ui_ps = psu.tile([P, P], f32)
            for c in range(E_CHUNKS):
                nc.tensor.matmul(
                    ui_ps,
                    lhsT=v_bc[:, c, :],
                    rhs=wsl_sb[:, c, :],
                    start=(c == 0),
                    stop=(c == E_CHUNKS - 1),
                )
            nc.vector.tensor_copy(ui_sb, ui_ps[0:1, :])
            nc.sync.dma_start(out=cc_in, in_=ui_sb)
            nc.gpsimd.collective_compute(
                "AllGather",
                mybir.AluOpType.bypass,
                ins=[cc_in[:, :]],
                outs=[cc_out[:, :]],
                replica_groups=[list(range(N_CORES))],
            )
            nc.sync.dma_start(out=u_sb, in_=cc_out[:, :])
            ub_ps = psu.tile([P, H], f32)
            for i in range(2):
                nc.tensor.matmul(
                    ub_ps[:, i * NH:(i + 1) * NH], lhsT=ones_row,
                    rhs=u_sb[:, i * NH:(i + 1) * NH],
                    start=True, stop=True,
                )
                nc.vector.tensor_copy(
                    u_rep[:, i * NH:(i + 1) * NH], ub_ps[:, i * NH:(i + 1) * NH]
                )
        scrp = ctx.enter_context(tc.tile_pool(name="scrp", bufs=2))
        ep = ctx.enter_context(tc.tile_pool(name="ep", bufs=2))
        small = ctx.enter_context(tc.tile_pool(name="small", bufs=2))
        encp = ctx.enter_context(tc.tile_pool(name="encp", bufs=28))
        roundp = ctx.enter_context(tc.tile_pool(name="roundp", bufs=8))
        psc = ctx.enter_context(tc.tile_pool(name="psc", bufs=2, space="PSUM"))
        psctx = ctx.enter_context(tc.tile_pool(name="psctx", bufs=2, space="PSUM"))

        for b in range(B_LOC):
            # Streamed pipeline per 128-position chunk:
            #   DMA chunk -> DVE dot with u -> ACT exp -> PE weighted-sum matmul.
            # exp() is applied unnormalized per chunk; the softmax denominator
            # is folded in at the end by scaling the context and weights.
            e_tile = ep.tile([P, S_CHUNKS], f32)
            exp_tile = ep.tile([P, S_CHUNKS], f32)
            expr_tile = ep.tile([P, S_CHUNKS], f32r)
            c_ps = [
                psctx.tile([1, NH], f32, name=f"cps{i}", tag=f"cps{i}")
                for i in range(2)
            ]
            for j in range(S_CHUNKS):
                t = encp.tile([P, H], f32)
                nc.sync.dma_start(out=t, in_=enc_d[b, j * P:(j + 1) * P, :])
                scr = scrp.tile([P, H], f32)
                nc.vector.scalar_tensor_tensor(
                    out=scr,
                    in0=t,
                    scalar=1.0,
                    in1=u_rep,
                    op0=mybir.AluOpType.mult,
                    op1=mybir.AluOpType.mult,
                    accum_out=e_tile[:, j:j + 1],
                )
                nc.scalar.activation(
                    out=exp_tile[:, j:j + 1], in_=e_tile[:, j:j + 1],
                    func=mybir.ActivationFunctionType.Exp,
                )
                # fp32r (TF32-like) weighted sum: 4x faster on the PE than
                # fp32. walrus requires fp32r matmul inputs to be produced by
                # a rounding op, so the copies get dedicated tiles.
                t_r = roundp.tile([P, H], f32r)
                nc.scalar.copy(t_r, t)
                nc.gpsimd.tensor_copy(
                    expr_tile[:, j:j + 1], exp_tile[:, j:j + 1]
                )
                for i in range(2):
                    nc.tensor.matmul(
                        c_ps[i],
                        lhsT=expr_tile[:, j:j + 1],
                        rhs=t_r[:, i * NH:(i + 1) * NH],
                        start=(j == 0),
                        stop=(j == S_CHUNKS - 1),
                    )

            # ---- softmax denominator + weights (no max shift; |e| << 88).
            # ones[128,16] lhsT makes the colsum land as [16,16] (same sum on
            # every partition), so one DVE reduce yields z16 [16,1] directly.
            zc_ps = psc.tile([S_CHUNKS, S_CHUNKS], f32, tag="zcps")
            expt_ps = psc.tile([S_CHUNKS, P], f32, tag="expt")
            nc.tensor.matmul(zc_ps, lhsT=ones_col, rhs=exp_tile,
                             start=True, stop=True)
            nc.tensor.transpose(expt_ps, exp_tile, ident)
            z16 = small.tile([S_CHUNKS, 1], f32)
            nc.vector.reduce_sum(z16, zc_ps, axis=mybir.AxisListType.X)
            rz16 = small.tile([S_CHUNKS, 1], f32)
            nc.vector.reciprocal(rz16, z16)
            w_t = small.tile([S_CHUNKS, P], f32)
            nc.vector.tensor_scalar_mul(w_t, in0=expt_ps, scalar1=rz16)
            nc.sync.dma_start(out=attnw_d[b], in_=w_t)
            r_z = rz16[0:1, 0:1]

            # ---- scale accumulated context by 1/Z ----
            ctx_sb = small.tile([1, H], f32)
            for i in range(2):
                nc.scalar.mul(ctx_sb[:, i * NH:(i + 1) * NH], c_ps[i], r_z)
            nc.sync.dma_start(out=ctx_d[b:b + 1, :], in_=ctx_sb)

    nc.compile()
    return nc


def _get_nc():
    if "nc" not in _CACHE:
        _CACHE["nc"] = _build_bass()
    return _CACHE["nc"]


def _run(in_maps, **kwargs):
    from concourse.bass_utils import run_bass_kernel_spmd

    nc = _get_nc()
    return run_bass_kernel_spmd(nc, in_maps, core_ids=list(range(N_CORES)), **kwargs)


def _make_in_maps(encoder_outputs, W, v):
    enc = np.ascontiguousarray(np.asarray(encoder_outputs, dtype=np.float32))
    w = np.asarray(W, dtype=np.float32)
    vv = np.ascontiguousarray(np.asarray(v, dtype=np.float32))
    assert enc.shape == (B, S, H) and w.shape == (H, 2 * H) and vv.shape == (H, 1)
    return [
        {
            "enc": enc[i * B_LOC:(i + 1) * B_LOC],
            "wslice": np.ascontiguousarray(w[:, H + i * P:H + (i + 1) * P]),
            "v": vv,
        }
        for i in range(N_CORES)
    ]


def _assemble(results):
    ctx = np.concatenate([r["ctx"] for r in results], axis=0)[None]
    attnw = np.concatenate(
        [r["attnw"].reshape(B_LOC, 1, S) for r in results], axis=0
    )
    return ctx.astype(np.float32), attnw.astype(np.float32)


def kernel(hidden_state=None, encoder_outputs=None, W=None, b=None, v=None,
           batch_size=None, sequence_length=None, **_unused):
    res = _run(_make_in_maps(encoder_outputs, W, v))
    return _assemble(res.results)


# revision 59
# speedup vs baseline: 1.0426x; 1.0426x over previous
"""Bahdanau attention Trainium2 kernel.

Math: the reference computes
    energy[b,s,:] = h[b] @ Wh.T + enc[b,s] @ We.T + bias          (Wh=W[:,:H], We=W[:,H:])
    attn_energies[b,s] = energy[b,s,:] @ v
    attn_weights = softmax_s(attn_energies)
    context[b] = attn_weights[b] @ enc[b]

Because attn_energies[b,s] = h[b]@(Wh.T@v) + enc[b,s]@(We.T@v) + bias@v and the
first/last terms are constant in s, they cancel inside the softmax.  So

    attn_weights[b] = softmax_s(enc[b] @ u),   u = We.T @ v   (computed on device)

which avoids the [B,S,H] energy tensor and its dense matmul entirely.  The
kernel streams each example's encoder block through SBUF once; per 128-row
chunk: DVE fused multiply-reduce against u, unshifted exp on ACT (max|e| ~ 66
so fp32 exp cannot overflow), and PE accumulation of the unnormalized
weighted sum, with 1/Z folded in at the end.  Energies and the softmax run
in full fp32; only the context weighted-sum matmul uses fp32r (TF32-like,
4x PE throughput), which costs ~2e-4 relative error on the context output.
To revert to full fp32 (~1.9e-5 error, ~8% slower): matmul on `t`/`exp_tile`
instead of `t_r`/`expr_tile` and drop the rounding copies.

Sharding: data-parallel over batch, 4 examples per core on 8 cores.
The only cross-core traffic is a 512B-per-rank AllGather of the u shards
(each core loads 1/8th of We instead of all 4MB of it).
"""

import numpy as np

B, S, H = 32, 2048, 1024
N_CORES = 8
B_LOC = B // N_CORES           # 4 examples per core
P = 128                        # SBUF partitions
S_CHUNKS = S // P              # 16 chunks of 128 positions
E_CHUNKS = H // P              # 8 chunks of the energy/contraction dim
NH = 512                       # matmul moving-free-dim limit

_CACHE = {}


def _build_bass():
    from contextlib import ExitStack

    import concourse.bacc as bacc
    import concourse.bass as bass
    import concourse.mybir as mybir
    from concourse.masks import make_identity
    from concourse.tile import TileContext

    f32 = mybir.dt.float32
    f32r = mybir.dt.float32r
    nc = bacc.Bacc(None, target_bir_lowering=False)

    enc_d = nc.declare_dram_parameter("enc", [B_LOC, S, H], f32, isOutput=False)
    w_d = nc.declare_dram_parameter("wslice", [H, P], f32, isOutput=False)
    v_d = nc.declare_dram_parameter("v", [H, 1], f32, isOutput=False)
    ctx_d = nc.declare_dram_parameter("ctx", [B_LOC, H], f32, isOutput=True)
    attnw_d = nc.declare_dram_parameter("attnw", [B_LOC, S_CHUNKS, P], f32, isOutput=True)

    with TileContext(nc) as tc, ExitStack() as ctx:
        const = ctx.enter_context(tc.tile_pool(name="const", bufs=1))
        wpool = ctx.enter_context(tc.tile_pool(name="wpool", bufs=1))

        ident = const.tile([P, P], f32)
        make_identity(nc, ident)
        ones_col = const.tile([P, S_CHUNKS], f32)
        nc.vector.memset(ones_col, 1.0)

        # v free-broadcast: v_bc[p, c, m] = v[c*128 + p] for all m, so a
        # single matmul chain produces u already broadcast across partitions:
        # ub[m, n] = sum_e v_bc[e, c, m] * We[e, n] = u[n] for every m.
        # (step-0 DMA APs are rejected by walrus, so broadcast on the DVE.)
        ones_sq = const.tile([P, P], f32)
        nc.vector.memset(ones_sq, 1.0)
        v_sb = const.tile([P, E_CHUNKS, 1], f32)
        nc.sync.dma_start(out=v_sb, in_=v_d[:, :].rearrange("(c p) o -> p c o", p=P))
        v_bc = const.tile([P, E_CHUNKS, P], f32)
        for c in range(E_CHUNKS):
            nc.vector.tensor_scalar_mul(
                v_bc[:, c, :], in0=ones_sq, scalar1=v_sb[:, c, :]
            )

        # Each core loads only ITS 128-column slice of We (512KB instead of
        # 4MB) and computes its u shard; a tiny AllGather (512B/rank) shares
        # the shards. wsl_sb[p, c, h'] = wslice[c*128 + p, h'].
        wsl_sb = wpool.tile([P, E_CHUNKS, P], f32)
        CG = E_CHUNKS // 2
        for g in range(2):
            nc.sync.dma_start(
                out=wsl_sb[:, g * CG:(g + 1) * CG, :],
                in_=w_d[g * CG * P:(g + 1) * CG * P, :].rearrange(
                    "(c p) h -> p c h", p=P),
            )

        # u_i[h'] = sum_e v[e] * wslice[e, h'] on this core, AllGather the 8
        # shards into u [1024], then broadcast across partitions on the PE.
        ones_row = const.tile([1, P], f32)
        nc.vector.memset(ones_row, 1.0)
        u_rep = const.tile([P, H], f32)
        u_sb = const.tile([1, H], f32)
        ui_sb = const.tile([1, P], f32)
        warm = const.tile([P, NH], f32)
        nc.vector.memset(warm, 0.0)
        dramp = ctx.enter_context(tc.tile_pool(name="dramp", bufs=1, space="DRAM"))
        cc_in = dramp.tile([1, P], f32)
        cc_out = dramp.tile([1, H], f32, addr_space="Shared")
        with tc.tile_pool(name="psu", bufs=1, space="PSUM") as psu:
            # Keep the PE continuously busy while the slice streams in, so the
            # p-state ramp reaches full clock before the real u matmuls.
            warm_ps = psu.tile([P, NH], f32)
            for _ in range(1):
                nc.tensor.matmul(warm_ps, lhsT=ident, rhs=warm,
                                 start=True, stop=True)
            ui_ps = psu.tile([P, P], f32)
            for c in range(E_CHUNKS):
                nc.tensor.matmul(
                    ui_ps,
                    lhsT=v_bc[:, c, :],
                    rhs=wsl_sb[:, c, :],
                    start=(c == 0),
                    stop=(c == E_CHUNKS - 1),
                )
            nc.vector.tensor_copy(ui_sb, ui_ps[0:1, :])
            nc.sync.dma_start(out=cc_in, in_=ui_sb)
            nc.gpsimd.collective_compute(
                "AllGather",
                mybir.AluOpType.bypass,
                ins=[cc_in[:, :]],
                outs=[cc_out[:, :]],
                replica_groups=[list(range(N_CORES))],
            )
            nc.sync.dma_start(out=u_sb, in_=cc_out[:, :])
            ub_ps = psu.tile([P, H], f32)
            for i in range(2):
                nc.tensor.matmul(
                    ub_ps[:, i * NH:(i + 1) * NH], lhsT=ones_row,
                    rhs=u_sb[:, i * NH:(i + 1) * NH],
                    start=True, stop=True,
                )
                nc.vector.tensor_copy(
                    u_rep[:, i * NH:(i + 1) * NH], ub_ps[:, i * NH:(i + 1) * NH]
                )
        scrp = ctx.enter_context(tc.tile_pool(name="scrp", bufs=2))
        ep = ctx.enter_context(tc.tile_pool(name="ep", bufs=2))
        small = ctx.enter_context(tc.tile_pool(name="small", bufs=2))
        encp = ctx.enter_context(tc.tile_pool(name="encp", bufs=28))
        roundp = ctx.enter_context(tc.tile_pool(name="roundp", bufs=8))
        psc = ctx.enter_context(tc.tile_pool(name="psc", bufs=2, space="PSUM"))
        psctx = ctx.enter_context(tc.tile_pool(name="psctx", bufs=2, space="PSUM"))

        for b in range(B_LOC):
            # Streamed pipeline per 128-position chunk:
            #   DMA chunk -> DVE dot with u -> ACT exp -> PE weighted-sum matmul.
            # exp() is applied unnormalized per chunk; the softmax denominator
            # is folded in at the end by scaling the context and weights.
            e_tile = ep.tile([P, S_CHUNKS], f32)
            exp_tile = ep.tile([P, S_CHUNKS], f32)
            expr_tile = ep.tile([P, S_CHUNKS], f32r)
            c_ps = [
                psctx.tile([1, NH], f32, name=f"cps{i}", tag=f"cps{i}")
                for i in range(2)
            ]
            for j in range(S_CHUNKS):
                t = encp.tile([P, H], f32)
                nc.sync.dma_start(out=t, in_=enc_d[b, j * P:(j + 1) * P, :])
                scr = scrp.tile([P, H], f32)
                nc.vector.scalar_tensor_tensor(
                    out=scr,
                    in0=t,
                    scalar=1.0,
                    in1=u_rep,
                    op0=mybir.AluOpType.mult,
                    op1=mybir.AluOpType.mult,
                    accum_out=e_tile[:, j:j + 1],
                )
                nc.scalar.activation(
                    out=exp_tile[:, j:j + 1], in_=e_tile[:, j:j + 1],
                    func=mybir.ActivationFunctionType.Exp,
                )
                # fp32r (TF32-like) weighted sum: 4x faster on the PE than
                # fp32. walrus requires fp32r matmul inputs to be produced by
                # a rounding op, so the copies get dedicated tiles.
                t_r = roundp.tile([P, H], f32r)
                nc.scalar.copy(t_r, t)
                nc.gpsimd.tensor_copy(
                    expr_tile[:, j:j + 1], exp_tile[:, j:j + 1]
                )
                for i in range(2):
                    nc.tensor.matmul(
                        c_ps[i],
                        lhsT=expr_tile[:, j:j + 1],
                        rhs=t_r[:, i * NH:(i + 1) * NH],
                        start=(j == 0),
                        stop=(j == S_CHUNKS - 1),
                    )

            # ---- softmax denominator + weights (no max shift; |e| << 88).
            # ones[128,16] lhsT makes the colsum land as [16,16] (same sum on
            # every partition), so one DVE reduce yields z16 [16,1] directly.
            zc_ps = psc.tile([S_CHUNKS, S_CHUNKS], f32, tag="zcps")
            expt_ps = psc.tile([S_CHUNKS, P], f32, tag="expt")
            nc.tensor.matmul(zc_ps, lhsT=ones_col, rhs=exp_tile,
                             start=True, stop=True)
            nc.tensor.transpose(expt_ps, exp_tile, ident)
            z16 = small.tile([S_CHUNKS, 1], f32)
            nc.vector.reduce_sum(z16, zc_ps, axis=mybir.AxisListType.X)
            rz16 = small.tile([S_CHUNKS, 1], f32)
            nc.vector.reciprocal(rz16, z16)
            w_t = small.tile([S_CHUNKS, P], f32)
            nc.vector.tensor_scalar_mul(w_t, in0=expt_ps, scalar1=rz16)
            nc.gpsimd.dma_start(out=attnw_d[b], in_=w_t)
            r_z = rz16[0:1, 0:1]

            # ---- scale accumulated context by 1/Z ----
            ctx_sb = small.tile([1, H], f32)
            for i in range(2):
                nc.scalar.mul(ctx_sb[:, i * NH:(i + 1) * NH], c_ps[i], r_z)
            nc.gpsimd.dma_start(out=ctx_d[b:b + 1, :], in_=ctx_sb)

    nc.compile()
    return nc


def _get_nc():
    if "nc" not in _CACHE:
        _CACHE["nc"] = _build_bass()
    return _CACHE["nc"]


def _run(in_maps, **kwargs):
    from concourse.bass_utils import run_bass_kernel_spmd

    nc = _get_nc()
    return run_bass_kernel_spmd(nc, in_maps, core_ids=list(range(N_CORES)), **kwargs)


def _make_in_maps(encoder_outputs, W, v):
    enc = np.ascontiguousarray(np.asarray(encoder_outputs, dtype=np.float32))
    w = np.asarray(W, dtype=np.float32)
    vv = np.ascontiguousarray(np.asarray(v, dtype=np.float32))
    assert enc.shape == (B, S, H) and w.shape == (H, 2 * H) and vv.shape == (H, 1)
    return [
        {
            "enc": enc[i * B_LOC:(i + 1) * B_LOC],
            "wslice": np.ascontiguousarray(w[:, H + i * P:H + (i + 1) * P]),
            "v": vv,
        }
        for i in range(N_CORES)
    ]


def _assemble(results):
    ctx = np.concatenate([r["ctx"] for r in results], axis=0)[None]
    attnw = np.concatenate(
        [r["attnw"].reshape(B_LOC, 1, S) for r in results], axis=0
    )
    return ctx.astype(np.float32), attnw.astype(np.float32)


def kernel(hidden_state=None, encoder_outputs=None, W=None, b=None, v=None,
           batch_size=None, sequence_length=None, **_unused):
    res = _run(_make_in_maps(encoder_outputs, W, v))
    return _assemble(res.results)
